# revision 38
# baseline (speedup 1.0000x reference)
"""Trainium2 Bass kernel for nn_Denoiser (dense MLP 2->16->16x5->2, N=4194304).

Strategy (pure data parallel over 8 NeuronCores):
  - Shard the batch over 8 cores (524288 points each); weights replicated.
  - On each core, stack 8 batch groups along SBUF partitions and use
    block-diagonal weights so each matmul column carries 8 points (the
    128x128 PE array runs 8 independent 16-wide MLPs at once).
  - float32r matmuls (1 cycle/row at N=512, vs 4 for emulated fp32) with
    fp32 PSUM accumulation; ~2e-3 scale-relative absmax error.
  - Activations are [128, 512] tiles (features x batch-columns); one
    matmul per layer (PSUM bank = 512 fp32 cols); the 6 inner ReLUs are
    fused into the PSUM->SBUF evacuation, split 50/50 between VectorE
    (tensor_scalar_max) and ScalarE (activation Relu). 7 single-bank
    PSUM slots + 5 software-pipelined super-chunk chains (B=5, emitted
    layer-step-interleaved) keep PE, VectorE and ScalarE all ~90% busy.
  - The final 16->2 layer accumulates 8 super-chunks into one packed
    [128, 512] PSUM tile (weight copies block-shifted by 16 partitions
    per super-chunk, PSUM-accumulated across the 8 matmuls) so a single
    PSUM->SBUF copy serves 8 super-chunks and the output DMA carries no
    padding. Measured ~300 us per core on TRN2 (8 cores in parallel).
  - The host pre-permutes x into the exact per-partition layout
    ([16, S*C], partition = 2*group+feature) so every device DMA is a
    contiguous 2D slice; the packed output [128, (S/8)*C] is decoded on
    the host the same way.
"""

import numpy as np

N = 4194304
N_CORES = 8
N_SHARD = N // N_CORES  # 524288
G = 8          # batch groups stacked along partitions
C = 512        # batch columns per super-chunk (1 PSUM bank per matmul)
S = N_SHARD // (G * C)  # 128 super-chunks per core
PACK = 8       # super-chunks packed per output evacuation (8 j-slots)
B = 6          # interleaved super-chunk chains (paired), software pipeline
N_NODE = 16
N_INT = 5

_CACHE = {}

# Set by test harnesses: TRACE=True captures an NTFF profile; the
# BassKernelResults of the last run lands in LAST_RESULT.
TRACE = False
LAST_RESULT = None


def _evac_engine(s, l):
    """True -> ScalarE (activation Relu), False -> VectorE (tensor_scalar_max).

    Measured on HW both cost ~690 ns at FD=512, so split evenly; the
    pack copy (1/8 per SC) alternates between the two engines.
    """
    return l % 2 == 1  # ACT gets {1,3,5}, DVE gets {0,2,4}


def _build_bass():
    from contextlib import ExitStack

    import concourse.mybir as mybir
    import concourse.tile as tile
    from concourse import bacc

    f32 = mybir.dt.float32
    f32r = mybir.dt.float32r
    nc = bacc.Bacc("TRN2", target_bir_lowering=False, num_devices=N_CORES)

    # xd[2g+f, s*C + c] = x[s*G*C + g*C + c, f]   (host pre-permuted)
    xd = nc.dram_tensor("xd", [16, S * C], f32r, kind="ExternalInput")
    w0 = nc.dram_tensor("w0", [16, 128], f32r, kind="ExternalInput")
    wm = nc.dram_tensor("wm", [N_INT, 128, 128], f32r, kind="ExternalInput")
    w6 = nc.dram_tensor("w6", [8, 128, 128], f32r, kind="ExternalInput")
    # yd[16*(s%PACK)+2g+f, (s//PACK)*C + c] = y[s*G*C + g*C + c, f]
    yd = nc.dram_tensor("yd", [128, (S // PACK) * C], f32, kind="ExternalOutput")

    with tile.TileContext(nc) as tc, ExitStack() as ctx:
        wpool = ctx.enter_context(tc.tile_pool(name="weights", bufs=1))
        xpool = ctx.enter_context(tc.tile_pool(name="x", bufs=14))
        hpool = ctx.enter_context(tc.tile_pool(name="h", bufs=16))
        opool = ctx.enter_context(tc.tile_pool(name="o", bufs=3))
        pspool = ctx.enter_context(tc.tile_pool(name="ps", bufs=1, space="PSUM"))
        pkpool = ctx.enter_context(tc.tile_pool(name="pk", bufs=1, space="PSUM"))

        w0_t = wpool.tile([16, 128], f32r, tag="w0")
        nc.sync.dma_start(out=w0_t, in_=w0[:, :])
        wm_t = []
        for l in range(N_INT):
            t = wpool.tile([128, 128], f32r, tag=f"wm{l}")
            (nc.sync if l % 2 == 0 else nc.scalar).dma_start(out=t, in_=wm[l, :, :])
            wm_t.append(t)
        w6_t = []
        for j in range(8):
            t = wpool.tile([128, 128], f32r, tag=f"w6{j}")
            w6_t.append(t)

        # 7-bank PSUM arena; slots rotate manually so paired (2-bank,
        # FD=1024) evacuations keep full 7-slot pipeline lookahead.
        # Tile tracks PSUM dependencies per bank within the tile.
        arena = pspool.tile([128, 7 * C], f32, tag="arena")
        slot = 0

        pk_t = None
        for b0 in range(0, S, B):
            blk = list(range(b0, min(b0 + B, S)))
            hs = []
            for k, s in enumerate(blk):
                x_t = xpool.tile([16, C], f32r, tag="x")
                nc.gpsimd.dma_start(out=x_t, in_=xd[:, s * C : (s + 1) * C])
                hs.append(x_t)
            if b0 == 0:
                # stream the output-layer weights behind the first x tiles;
                # they are not needed until the first pack matmul.
                for j in range(8):
                    (nc.scalar if j % 2 == 0 else nc.sync).dma_start(
                        out=w6_t[j], in_=w6[j, :, :]
                    )
            for l in range(6):
                lhsT = w0_t if l == 0 else wm_t[l - 1]
                new_hs = []
                for p in range(len(blk) // 2):
                    a = slot
                    b = (slot + 1) % 7
                    slot = (slot + 2) % 7
                    nc.tensor.matmul(
                        arena[:, a * C : (a + 1) * C], lhsT, hs[2 * p],
                        start=True, stop=True,
                    )
                    nc.tensor.matmul(
                        arena[:, b * C : (b + 1) * C], lhsT, hs[2 * p + 1],
                        start=True, stop=True,
                    )
                    h2 = hpool.tile([128, 2 * C], f32r, tag="h")
                    dve_first = (l + p) % 2 == 0
                    if b == a + 1:
                        if dve_first:
                            nc.vector.tensor_scalar_max(
                                h2, arena[:, a * C : (a + 2) * C], 0.0
                            )
                        else:
                            nc.scalar.activation(
                                h2, arena[:, a * C : (a + 2) * C],
                                mybir.ActivationFunctionType.Relu,
                            )
                    else:
                        # slot pair wraps the arena end: two FD=512 ops,
                        # one per engine.
                        nc.vector.tensor_scalar_max(
                            h2[:, 0:C], arena[:, a * C : (a + 1) * C], 0.0
                        )
                        nc.scalar.activation(
                            h2[:, C : 2 * C], arena[:, b * C : (b + 1) * C],
                            mybir.ActivationFunctionType.Relu,
                        )
                    new_hs.append(h2[:, 0:C])
                    new_hs.append(h2[:, C : 2 * C])
                hs = new_hs
            for i, s in enumerate(blk):
                j = s % PACK
                if j == 0:
                    pk_t = pkpool.tile([128, C], f32, tag="pk")
                nc.tensor.matmul(
                    pk_t,
                    w6_t[j],
                    hs[i],
                    start=(j == 0),
                    stop=(j == PACK - 1),
                    skip_group_check=True,
                )
                if j == PACK - 1:
                    sp = s // PACK
                    o_t = opool.tile([128, C], f32, tag="o")
                    nc.vector.tensor_copy(o_t, pk_t)
                    nc.sync.dma_start(out=yd[:, sp * C : (sp + 1) * C], in_=o_t)
    nc.compile()
    return nc


def _prep_weights(w_in, w_mid, w_out):
    """Block-diagonal stationary operands (lhsT = W.T blocks) for 8 groups."""
    w0 = np.zeros((16, 128), dtype=np.float32)
    for g in range(G):
        w0[2 * g : 2 * g + 2, 16 * g : 16 * g + 16] = w_in.T  # [2,16]
    wm = np.zeros((N_INT, 128, 128), dtype=np.float32)
    for l in range(N_INT):
        for g in range(G):
            wm[l, 16 * g : 16 * g + 16, 16 * g : 16 * g + 16] = w_mid[l].T
    w6 = np.zeros((8, 128, 128), dtype=np.float32)
    for j in range(8):
        for g in range(G):
            w6[j, 16 * g : 16 * g + 16, 16 * j + 2 * g : 16 * j + 2 * g + 2] = (
                w_out.T
            )  # [16,2]
    return w0, wm, w6


def _shard_x(shard):
    """[N_SHARD, 2] -> [16, S*C] with row 2g+f, col s*C+c."""
    v = shard.reshape(S, G, C, 2)           # [s, g, c, f]
    v = v.transpose(1, 3, 0, 2)             # [g, f, s, c]
    return np.ascontiguousarray(v.reshape(16, S * C))


def _unshard_y(yd):
    """[128, (S//PACK)*C] -> [N_SHARD, 2].  Row q = 16*j + 2*g + f."""
    v = yd.reshape(PACK, 8, 2, S // PACK, C)          # [j, g, f, sp, c]
    v = v.transpose(3, 0, 1, 4, 2)                    # [sp, j, g, c, f]
    return v.reshape(N_SHARD, 2)


def kernel(x, w_in, w_mid, w_out):
    from concourse.bass_utils import run_bass_kernel_spmd

    x = np.ascontiguousarray(x, dtype=np.float32)
    w0, wm, w6 = _prep_weights(
        np.asarray(w_in, dtype=np.float32),
        np.asarray(w_mid, dtype=np.float32),
        np.asarray(w_out, dtype=np.float32),
    )

    if "nc" not in _CACHE:
        _CACHE["nc"] = _build_bass()
    nc = _CACHE["nc"]

    in_maps = []
    for c in range(N_CORES):
        shard = x[c * N_SHARD : (c + 1) * N_SHARD]
        in_maps.append({"xd": _shard_x(shard), "w0": w0, "wm": wm, "w6": w6})

    res = run_bass_kernel_spmd(
        nc, in_maps, core_ids=list(range(N_CORES)), trace=TRACE
    )
    global LAST_RESULT
    LAST_RESULT = res
    y = np.empty((N, 2), dtype=np.float32)
    for c in range(N_CORES):
        y[c * N_SHARD : (c + 1) * N_SHARD] = _unshard_y(res.results[c]["yd"])
    return y


# revision 39
# speedup vs baseline: 3.4544x; 3.4544x over previous
"""Trainium2 Bass kernel for nn_Denoiser (dense MLP 2->16->16x5->2, N=4194304).

Strategy (pure data parallel over 8 NeuronCores):
  - Shard the batch over 8 cores (524288 points each); weights replicated.
  - On each core, stack 8 batch groups along SBUF partitions and use
    block-diagonal weights so each matmul column carries 8 points (the
    128x128 PE array runs 8 independent 16-wide MLPs at once).
  - float32r matmuls (1 cycle/row at N=512, vs 4 for emulated fp32) with
    fp32 PSUM accumulation; ~2e-3 scale-relative absmax error.
  - Activations are [128, 512] tiles (features x batch-columns); one
    matmul per layer (PSUM bank = 512 fp32 cols); the 6 inner ReLUs are
    fused into the PSUM->SBUF evacuation, split 50/50 between VectorE
    (tensor_scalar_max) and ScalarE (activation Relu). 7 single-bank
    PSUM slots + 5 software-pipelined super-chunk chains (B=5, emitted
    layer-step-interleaved) keep PE, VectorE and ScalarE all ~90% busy.
  - The final 16->2 layer accumulates 8 super-chunks into one packed
    [128, 512] PSUM tile (weight copies block-shifted by 16 partitions
    per super-chunk, PSUM-accumulated across the 8 matmuls) so a single
    PSUM->SBUF copy serves 8 super-chunks and the output DMA carries no
    padding. Measured ~300 us per core on TRN2 (8 cores in parallel).
  - The host pre-permutes x into the exact per-partition layout
    ([16, S*C], partition = 2*group+feature) so every device DMA is a
    contiguous 2D slice; the packed output [128, (S/8)*C] is decoded on
    the host the same way.
"""

import numpy as np

N = 4194304
N_CORES = 8
N_SHARD = N // N_CORES  # 524288
G = 8          # batch groups stacked along partitions
C = 512        # batch columns per super-chunk (1 PSUM bank per matmul)
S = N_SHARD // (G * C)  # 128 super-chunks per core
PACK = 8       # super-chunks packed per output evacuation (8 j-slots)
B = 5          # interleaved super-chunk chains, software pipeline width
N_NODE = 16
N_INT = 5

_CACHE = {}

# Set by test harnesses: TRACE=True captures an NTFF profile; the
# BassKernelResults of the last run lands in LAST_RESULT.
TRACE = False
LAST_RESULT = None


def _evac_engine(s, l):
    """True -> ScalarE (activation Relu), False -> VectorE (tensor_scalar_max).

    Measured on HW both cost ~690 ns at FD=512, so split evenly; the
    pack copy (1/8 per SC) alternates between the two engines.
    """
    return l % 2 == 1  # ACT gets {1,3,5}, DVE gets {0,2,4}


def _build_bass():
    from contextlib import ExitStack

    import concourse.mybir as mybir
    import concourse.tile as tile
    from concourse import bacc

    f32 = mybir.dt.float32
    f32r = mybir.dt.float32r
    nc = bacc.Bacc("TRN2", target_bir_lowering=False, num_devices=N_CORES)

    # xd[2g+f, s*C + c] = x[s*G*C + g*C + c, f]   (host pre-permuted)
    xd = nc.dram_tensor("xd", [16, S * C], f32r, kind="ExternalInput")
    w0 = nc.dram_tensor("w0", [16, 128], f32r, kind="ExternalInput")
    wm = nc.dram_tensor("wm", [N_INT, 128, 128], f32r, kind="ExternalInput")
    w6 = nc.dram_tensor("w6", [8, 128, 128], f32r, kind="ExternalInput")
    # yd[16*(s%PACK)+2g+f, (s//PACK)*C + c] = y[s*G*C + g*C + c, f]
    yd = nc.dram_tensor("yd", [128, (S // PACK) * C], f32, kind="ExternalOutput")

    with tile.TileContext(nc) as tc, ExitStack() as ctx:
        wpool = ctx.enter_context(tc.tile_pool(name="weights", bufs=1))
        xpool = ctx.enter_context(tc.tile_pool(name="x", bufs=14))
        hpool = ctx.enter_context(tc.tile_pool(name="h", bufs=16))
        opool = ctx.enter_context(tc.tile_pool(name="o", bufs=3))
        pspool = ctx.enter_context(tc.tile_pool(name="ps", bufs=7, space="PSUM"))
        pkpool = ctx.enter_context(tc.tile_pool(name="pk", bufs=1, space="PSUM"))

        w0_t = wpool.tile([16, 128], f32r, tag="w0")
        nc.sync.dma_start(out=w0_t, in_=w0[:, :])
        wm_t = []
        for l in range(N_INT):
            t = wpool.tile([128, 128], f32r, tag=f"wm{l}")
            (nc.sync if l % 2 == 0 else nc.scalar).dma_start(out=t, in_=wm[l, :, :])
            wm_t.append(t)
        w6_t = []
        for j in range(8):
            t = wpool.tile([128, 128], f32r, tag=f"w6{j}")
            w6_t.append(t)

        pk_t = None
        for b0 in range(0, S, B):
            blk = list(range(b0, min(b0 + B, S)))
            hs = []
            for k, s in enumerate(blk):
                x_t = xpool.tile([16, C], f32r, tag="x")
                nc.gpsimd.dma_start(out=x_t, in_=xd[:, s * C : (s + 1) * C])
                hs.append(x_t)
            if b0 == 0:
                # stream the output-layer weights behind the first x tiles;
                # they are not needed until the first pack matmul.
                for j in range(8):
                    (nc.scalar if j % 2 == 0 else nc.sync).dma_start(
                        out=w6_t[j], in_=w6[j, :, :]
                    )
            for l in range(6):
                lhsT = w0_t if l == 0 else wm_t[l - 1]
                new_hs = []
                for i in range(len(blk)):
                    ps_t = pspool.tile([128, C], f32, tag="ps")
                    nc.tensor.matmul(ps_t, lhsT, hs[i], start=True, stop=True)
                    h_new = hpool.tile([128, C], f32r, tag="h")
                    if (l + i) % 2 == 1:
                        nc.scalar.activation(
                            h_new, ps_t, mybir.ActivationFunctionType.Relu
                        )
                    else:
                        nc.vector.tensor_scalar_max(h_new, ps_t, 0.0)
                    new_hs.append(h_new)
                hs = new_hs
            for i, s in enumerate(blk):
                j = s % PACK
                if j == 0:
                    pk_t = pkpool.tile([128, C], f32, tag="pk")
                nc.tensor.matmul(
                    pk_t,
                    w6_t[j],
                    hs[i],
                    start=(j == 0),
                    stop=(j == PACK - 1),
                    skip_group_check=True,
                )
                if j == PACK - 1:
                    sp = s // PACK
                    o_t = opool.tile([128, C], f32, tag="o")
                    nc.vector.tensor_copy(o_t, pk_t)
                    nc.sync.dma_start(out=yd[:, sp * C : (sp + 1) * C], in_=o_t)
    nc.compile()
    return nc


def _prep_weights(w_in, w_mid, w_out):
    """Block-diagonal stationary operands (lhsT = W.T blocks) for 8 groups."""
    w0 = np.zeros((16, 128), dtype=np.float32)
    for g in range(G):
        w0[2 * g : 2 * g + 2, 16 * g : 16 * g + 16] = w_in.T  # [2,16]
    wm = np.zeros((N_INT, 128, 128), dtype=np.float32)
    for l in range(N_INT):
        for g in range(G):
            wm[l, 16 * g : 16 * g + 16, 16 * g : 16 * g + 16] = w_mid[l].T
    w6 = np.zeros((8, 128, 128), dtype=np.float32)
    for j in range(8):
        for g in range(G):
            w6[j, 16 * g : 16 * g + 16, 16 * j + 2 * g : 16 * j + 2 * g + 2] = (
                w_out.T
            )  # [16,2]
    return w0, wm, w6


def _shard_x(shard):
    """[N_SHARD, 2] -> [16, S*C] with row 2g+f, col s*C+c."""
    v = shard.reshape(S, G, C, 2)           # [s, g, c, f]
    v = v.transpose(1, 3, 0, 2)             # [g, f, s, c]
    return np.ascontiguousarray(v.reshape(16, S * C))


def _unshard_y(yd):
    """[128, (S//PACK)*C] -> [N_SHARD, 2].  Row q = 16*j + 2*g + f."""
    v = yd.reshape(PACK, 8, 2, S // PACK, C)          # [j, g, f, sp, c]
    v = v.transpose(3, 0, 1, 4, 2)                    # [sp, j, g, c, f]
    return v.reshape(N_SHARD, 2)


def kernel(x, w_in, w_mid, w_out):
    from concourse.bass_utils import run_bass_kernel_spmd

    x = np.ascontiguousarray(x, dtype=np.float32)
    w0, wm, w6 = _prep_weights(
        np.asarray(w_in, dtype=np.float32),
        np.asarray(w_mid, dtype=np.float32),
        np.asarray(w_out, dtype=np.float32),
    )

    if "nc" not in _CACHE:
        _CACHE["nc"] = _build_bass()
    nc = _CACHE["nc"]

    in_maps = []
    for c in range(N_CORES):
        shard = x[c * N_SHARD : (c + 1) * N_SHARD]
        in_maps.append({"xd": _shard_x(shard), "w0": w0, "wm": wm, "w6": w6})

    res = run_bass_kernel_spmd(
        nc, in_maps, core_ids=list(range(N_CORES)), trace=TRACE
    )
    global LAST_RESULT
    LAST_RESULT = res
    y = np.empty((N, 2), dtype=np.float32)
    for c in range(N_CORES):
        y[c * N_SHARD : (c + 1) * N_SHARD] = _unshard_y(res.results[c]["yd"])
    return y
